# revision 1
# baseline (speedup 1.0000x reference)
"""Trainium2 Bass kernel for y = x @ W^T + b  (4096x4096 @ 4096x4096 + 4096).

Sharding: data-parallel over batch. Core c gets x rows [c*512:(c+1)*512];
W and b are replicated. Each core computes yT_c = W @ x_c^T + b[:, None]
(output transposed, [4096, 512]) and the host reassembles
y = concat([yT_c.T for c in cores], axis=0). No collectives.

Per-core kernel (bf16 compute, fp32 accumulate in PSUM):
  - x_c: SWDGE DMA-cast f32->bf16 into SBUF natural (one 3D-AP DMA),
    transposed on the PE (128x128 transpose-mode blocks) into resident
    xT [128, 32, 512].
  - W: per 128-row slab, SWDGE DMA-cast f32->bf16, PE transpose-mode
    blocks -> PSUM (GK=8 blocks per bank), DVE eviction
    to wT [128, 32, 128]; then 32 accumulating matmuls (lhsT=wT block,
    rhs=xT block, N=512) into one PSUM bank (4 accum + 4 transpose banks).
  - PSUM eviction fused with bias add on ScalarE, DMA out on sync.

Measured (min of 3, whole-NEFF neuron-profile exec_time_ns): ~325 us,
relative error ~2.0e-3 vs the fp32 reference.
"""

import os
import sys

for _p in ("/opt/trn_rl_repo", "/opt/pypackages"):
    if _p not in sys.path and os.path.isdir(_p):
        sys.path.append(_p)

import numpy as np

import concourse.bass as bass
import concourse.tile as tile
from concourse import bacc, mybir
from concourse.bass_utils import run_bass_kernel_spmd

N_CORES = 8
BATCH = 4096
IN_F = 4096
OUT_F = 4096
P = 128
B = BATCH // N_CORES          # 512 batch rows per core
KT = IN_F // P                # 32 contraction tiles
OT = OUT_F // P               # 32 output-feature tiles

_F32 = mybir.dt.float32
_BF16 = mybir.dt.bfloat16

_compiled_nc = None


def _build():
    nc = bacc.Bacc("TRN2", target_bir_lowering=False, debug=False,
                   num_devices=N_CORES)

    x = nc.dram_tensor("x", [B, IN_F], _F32, kind="ExternalInput")
    w = nc.dram_tensor("weight", [OUT_F, IN_F], _F32, kind="ExternalInput")
    bias = nc.dram_tensor("bias", [OUT_F], _F32, kind="ExternalInput")
    out = nc.dram_tensor("out", [OUT_F, B], _F32, kind="ExternalOutput")

    from concourse.masks import make_identity

    GK = 8                     # k-tiles transposed per PSUM bank batch
    WH = 1                     # W cast split per slab
    BT = B // P                # 4 batch tiles
    with tile.TileContext(nc) as tc:
        with tc.tile_pool(name="const", bufs=1) as const, \
             tc.tile_pool(name="wnat", bufs=2 * WH) as wnat_pool, \
             tc.tile_pool(name="wt", bufs=3) as wt_pool, \
             tc.tile_pool(name="tpsum", bufs=4, space="PSUM") as tpsum_pool, \
             tc.tile_pool(name="psum", bufs=4, space="PSUM") as psum_pool, \
             tc.tile_pool(name="yout", bufs=3) as y_pool:

            ident = const.tile([P, P], _BF16)
            make_identity(nc, ident)
            ident32 = const.tile([32, 32], _F32)
            make_identity(nc, ident32)

            # ---- bias: one DMA to [32, 128], PE transpose -> [128, 32]
            b_nat = const.tile([32, P], _F32)
            nc.scalar.dma_start(out=b_nat[:],
                                in_=bias[:].rearrange("(a b) -> a b", b=P))
            b_ps = tpsum_pool.tile([P, 32], _F32, name="b_ps", tag="tps")
            nc.tensor.transpose(b_ps[:], b_nat[:], ident32[:])
            bias_sb = const.tile([P, OT], _F32)
            nc.vector.tensor_copy(out=bias_sb[:], in_=b_ps[:])

            # ---- x: DMA-cast to SBUF natural, PE-transpose to xT (resident)
            x_nat = const.tile([P, BT, IN_F], _BF16)
            nc.gpsimd.dma_start(
                out=x_nat[:],
                in_=x[:, :].rearrange("(bt p) i -> p bt i", p=P))
            xT = const.tile([P, KT, B], _BF16)
            for kt in range(KT):
                pst = tpsum_pool.tile([P, BT, P], _BF16, name=f"xps{kt}",
                                      tag="tps")
                for bt in range(BT):
                    nc.tensor.transpose(pst[:, bt, :],
                                        x_nat[:, bt, kt * P:(kt + 1) * P],
                                        ident[:])
                nc.vector.tensor_copy(out=xT[:, kt, :], in_=pst[:])

            # ---- main loop over output-feature tiles
            IH = IN_F // WH
            KH = IH // P   # k-tiles per half-slab
            for ot in range(OT):
                w_nat = [wnat_pool.tile([P, IH], _BF16, tag=f"wnat{h}",
                                        name=f"wnat{h}_{ot}")
                         for h in range(WH)]
                for h in range(WH):
                    nc.gpsimd.dma_start(
                        out=w_nat[h][:],
                        in_=w[ot * P:(ot + 1) * P, h * IH:(h + 1) * IH])

                # Transpose slab on the PE (transpose-mode), GK blocks per
                # PSUM bank, DVE-evicted per bank.
                wT = wt_pool.tile([P, KT, P], _BF16)
                for g in range(KT // GK):
                    pst = tpsum_pool.tile([P, GK, P], _BF16, tag="tps",
                                          name=f"wps_{ot}_{g}")
                    for j in range(GK):
                        kt = g * GK + j
                        src = w_nat[kt // KH]
                        k0 = (kt % KH) * P
                        nc.tensor.transpose(pst[:, j, :],
                                            src[:, k0:k0 + P],
                                            ident[:])
                    nc.vector.tensor_copy(out=wT[:, g * GK:(g + 1) * GK, :],
                                          in_=pst[:])

                ps = psum_pool.tile([P, B], _F32)
                for kt in range(KT):
                    nc.tensor.matmul(ps[:], lhsT=wT[:, kt, :],
                                     rhs=xT[:, kt, :],
                                     start=(kt == 0), stop=(kt == KT - 1))

                ysb = y_pool.tile([P, B], _F32)
                nc.scalar.activation(ysb[:], ps[:],
                                     mybir.ActivationFunctionType.Identity,
                                     bias=bias_sb[:, ot:ot + 1])
                nc.sync.dma_start(out=out[ot * P:(ot + 1) * P, :], in_=ysb[:])

    nc.compile()
    return nc


def _get_nc():
    global _compiled_nc
    if _compiled_nc is None:
        _compiled_nc = _build()
    return _compiled_nc


def _run(inputs, trace=False, trace_cores=None):
    x = np.ascontiguousarray(np.asarray(inputs["x"], dtype=np.float32))
    w = np.ascontiguousarray(np.asarray(inputs["weight"], dtype=np.float32))
    b = np.ascontiguousarray(np.asarray(inputs["bias"], dtype=np.float32))

    nc = _get_nc()
    in_maps = [
        {"x": x[c * B:(c + 1) * B], "weight": w, "bias": b}
        for c in range(N_CORES)
    ]
    res = run_bass_kernel_spmd(nc, in_maps, core_ids=list(range(N_CORES)),
                               trace=trace, trace_cores=trace_cores)
    y = np.concatenate([res.results[c]["out"].T for c in range(N_CORES)], axis=0)
    return y, res


def kernel(**inputs):
    y, _ = _run(inputs)
    return y



# revision 4
# speedup vs baseline: 1.5451x; 1.5451x over previous
"""Trainium2 Bass kernel for y = x @ W^T + b  (4096x4096 @ 4096x4096 + 4096).

Sharding: 2D grid, R=4 batch-groups x C=2 out-feature-groups. Core (r, c)
computes yT_rc = W_c @ x_r^T + b_c[:, None]  ([2048, 1024], output
transposed) and the host reassembles y. No collectives.

All layout work happens on the host: x and W slices are transposed,
tiled to the exact SBUF layout, and cast to bf16 in numpy. The device
kernel is nothing but back-to-back bf16 matmuls (fp32 PSUM accumulate):

  - xT_r [128, 32*1024] bf16 resident in SBUF (one partition-contiguous
    DMA, split in 8 chunks so compute starts early).
  - per o-tile (16): W slab [128, 32*128] bf16 DMA (double-buffered),
    32 k-tiles x 2 batch-chunk matmuls (N=512) accumulating in PSUM,
    ScalarE eviction fused with bias add, HWDGE DMA out.

PE roofline: 1024 MM x 512 cols / 2.4 GHz = 218.5 us per core.
"""

import os
import sys

for _p in ("/opt/trn_rl_repo", "/opt/pypackages"):
    if _p not in sys.path and os.path.isdir(_p):
        sys.path.append(_p)

import numpy as np
import ml_dtypes

import concourse.bass as bass
import concourse.tile as tile
from concourse import bacc, mybir
from concourse.bass_utils import run_bass_kernel_spmd

N_CORES = 8
R = 4                          # batch groups
C = 2                          # out-feature groups
BATCH = 4096
IN_F = 4096
OUT_F = 4096
P = 128
BR = BATCH // R                # 1024 batch rows per core
OC = OUT_F // C                # 2048 out features per core
KT = IN_F // P                 # 32 contraction tiles
OT = OC // P                   # 16 output-feature tiles per core
NB = BR // 512                 # 2 psum-width batch chunks

_F32 = mybir.dt.float32
_BF16 = mybir.dt.bfloat16
_BF16_NP = ml_dtypes.bfloat16

_compiled_nc = None


def _build():
    nc = bacc.Bacc("TRN2", target_bir_lowering=False, debug=False,
                   num_devices=N_CORES)

    # Host-pretiled layouts (see _prep_inputs):
    #   xt[p, it*BR + b]   = x_r[b, it*128 + p]      (bf16)
    #   wt[ot*128 + p, it*128 + o2] = w_c[ot*128 + o2, it*128 + p]  (bf16)
    xt = nc.dram_tensor("xt", [P, KT * BR], _BF16, kind="ExternalInput")
    wt = nc.dram_tensor("wt", [OT * P, KT * P], _BF16, kind="ExternalInput")
    bias = nc.dram_tensor("bias", [OC], _F32, kind="ExternalInput")
    out = nc.dram_tensor("out", [OC, BR], _F32, kind="ExternalOutput")

    from concourse.masks import make_identity

    XCH = 8                    # x resident load split into 8 chunks
    with tile.TileContext(nc) as tc:
        with tc.tile_pool(name="const", bufs=1) as const, \
             tc.tile_pool(name="wslab", bufs=3) as wpool, \
             tc.tile_pool(name="bps", bufs=1, space="PSUM") as bps_pool, \
             tc.tile_pool(name="psum", bufs=4, space="PSUM") as pspool, \
             tc.tile_pool(name="yout", bufs=3) as ypool:

            # ---- bias: one DMA to [OT, 128], PE transpose -> [128, OT]
            identN = const.tile([OT, OT], _F32)
            make_identity(nc, identN)
            b_nat = const.tile([OT, P], _F32)
            nc.scalar.dma_start(out=b_nat[:],
                                in_=bias[:].rearrange("(a b) -> a b", b=P))
            b_ps = bps_pool.tile([P, OT], _F32)
            nc.tensor.transpose(b_ps[:], b_nat[:], identN[:])
            bias_sb = const.tile([P, OT], _F32)
            nc.vector.tensor_copy(out=bias_sb[:], in_=b_ps[:])

            # ---- x: resident in SBUF, straight partition-contiguous DMA
            x_sb = const.tile([P, KT * BR], _BF16)
            xc = (KT * BR) // XCH
            for g in range(XCH):
                nc.gpsimd.dma_start(out=x_sb[:, g * xc:(g + 1) * xc],
                                    in_=xt[:, g * xc:(g + 1) * xc])

            # ---- main loop over output-feature tiles
            for ot in range(OT):
                w_sb = wpool.tile([P, KT * P], _BF16, name=f"w{ot}", tag="w")
                nc.sync.dma_start(out=w_sb[:],
                                  in_=wt[ot * P:(ot + 1) * P, :])

                y_sb = ypool.tile([P, BR], _F32, name=f"y{ot}", tag="y")
                for j in range(NB):
                    ps = pspool.tile([P, 512], _F32, name=f"ps{ot}_{j}",
                                     tag="ps")
                    for it in range(KT):
                        b0 = it * BR + j * 512
                        nc.tensor.matmul(ps[:],
                                         lhsT=w_sb[:, it * P:(it + 1) * P],
                                         rhs=x_sb[:, b0:b0 + 512],
                                         start=(it == 0), stop=(it == KT - 1))
                    nc.scalar.activation(y_sb[:, j * 512:(j + 1) * 512], ps[:],
                                         mybir.ActivationFunctionType.Identity,
                                         bias=bias_sb[:, ot:ot + 1])
                nc.scalar.dma_start(out=out[ot * P:(ot + 1) * P, :], in_=y_sb[:])

    nc.compile()
    return nc


def _get_nc():
    global _compiled_nc
    if _compiled_nc is None:
        _compiled_nc = _build()
    return _compiled_nc


def _prep_inputs(inputs):
    x = np.ascontiguousarray(np.asarray(inputs["x"], dtype=np.float32))
    w = np.ascontiguousarray(np.asarray(inputs["weight"], dtype=np.float32))
    b = np.ascontiguousarray(np.asarray(inputs["bias"], dtype=np.float32))

    # x tiles per batch group r: [p, it*BR + b] = x_r[b, it*128 + p]
    xts = []
    for r in range(R):
        xs = x[r * BR:(r + 1) * BR, :]                      # [BR, IN_F]
        xt = xs.T.reshape(KT, P, BR).transpose(1, 0, 2)     # [P, KT, BR]
        xts.append(np.ascontiguousarray(
            xt.astype(_BF16_NP).reshape(P, KT * BR)))

    # W tiles per out-feature group c:
    # [ot*128 + p, it*128 + o2] = w_c[ot*128 + o2, it*128 + p]
    wts, bs = [], []
    for c in range(C):
        ws = w[c * OC:(c + 1) * OC, :]                      # [OC, IN_F]
        wtt = ws.T.reshape(KT, P, OT, P).transpose(2, 1, 0, 3)  # [OT,P,KT,P]
        wts.append(np.ascontiguousarray(
            wtt.astype(_BF16_NP).reshape(OT * P, KT * P)))
        bs.append(np.ascontiguousarray(b[c * OC:(c + 1) * OC]))

    in_maps = []
    for core in range(N_CORES):
        r, c = divmod(core, C)
        in_maps.append({"xt": xts[r], "wt": wts[c], "bias": bs[c]})
    return in_maps


def _run(inputs, trace=False, trace_cores=None):
    nc = _get_nc()
    in_maps = _prep_inputs(inputs)
    res = run_bass_kernel_spmd(nc, in_maps, core_ids=list(range(N_CORES)),
                               trace=trace, trace_cores=trace_cores)
    y = np.empty((BATCH, OUT_F), dtype=np.float32)
    for core in range(N_CORES):
        r, c = divmod(core, C)
        y[r * BR:(r + 1) * BR, c * OC:(c + 1) * OC] = res.results[core]["out"].T
    return y, res


def kernel(**inputs):
    y, _ = _run(inputs)
    return y
